# revision 1
# baseline (speedup 1.0000x reference)
"""Batch-parallel dot-product attention for TRN2 (8 NeuronCores).

reference: context[b] = softmax(Q[b] @ K[b].T / sqrt(64)) @ V[b]
with Q,K,V: [32, 2048, 64] fp32.

Sharding: pure data parallel - 4 batches per core, no collectives.

Per-core kernel, per (batch, 1024-query half), 16 key-tile steps:
  sc[k, q]  = (K_t @ Q^T)/1      one 1024-wide matmul per k-tile (PE)
  pt        = exp(sc/8) fp16     ACT for most steps; DVE bit-trick exp2
                                 for DVE_KS steps to offload the ACT
                                 bottleneck (ACT is 1 elem/lane/cycle)
  cx[d, q] += Vaug_t^T @ pt      PSUM accumulation, Vaug = [V | 1]
  (row 64 of cx = softmax denominator via the ones column)
  drain: copy cx->SBUF, transpose 128-col chunks on PE, reciprocal of
  denom, per-chunk scalar-ptr mul, DMA out in [q, 8*64] layout.

AV matmuls are deferred (emitted after the NEXT step's QK matmul) so PE
never waits on the exp producing their rhs; DVE-step AVs defer 4 steps
to cover the longer DVE chain latency.
"""

import numpy as np

import concourse.bass as bass
import concourse.bacc as bacc
import concourse.tile as tile
from concourse import mybir
from concourse.bass_utils import run_bass_kernel_spmd

NCORES = 8
BPC = 4  # batches per core
S = 2048
D = 64
DA = 96  # V augmented to 96 cols (64 V + 32 ones) for 32-aligned xbar transpose
NKT = S // 128  # 16 key tiles of 128
NH = 2  # query halves
HQ = S // NH  # 1024 queries per half

# steps (k-tile indices) per (b,h) whose exp runs on DVE instead of ACT
DVE_KS = (4, 9)


def dve_steps(b, h):
    return DVE_KS
# extra deferral (in steps) for AV matmuls of DVE steps
DVE_AV_DELAY = 6

FP16 = mybir.dt.float16
F32 = mybir.dt.float32
I16 = mybir.dt.int16

# fp16 exp2 bit-trick constants (DVE offload path): exp(s/8) = 2^y,
# y = s*log2(e)/8; t = fp16(y + 1536) rounds y to int n = t - 1536;
# s16 = t*1024 + SBIAS = fp16 bits of 2^n; f = y - n in [-0.5, 0.5];
# 2^f ~ C0 + C1*f + C2*f^2 (minimax, rel err 2.2e-3).
LOG2E_8 = 1.4426950408889634 / 8.0
MAGIC = 1536.0
SBIAS = 15360.0 - MAGIC * 1024.0
C0, C1, C2 = 1.00053068, 0.70552215, 0.23946112

_cache = {}


def _build(reps=1):
    if reps in _cache:
        return _cache[reps]

    nc = bacc.Bacc(
        "TRN2",
        target_bir_lowering=False,
        debug=False,
        num_devices=1,
        enable_partition_id=False,
    )

    qt_d = nc.dram_tensor("qt", [BPC, D, S], FP16, kind="ExternalInput").ap()
    kt_d = nc.dram_tensor("kt", [BPC, D, S], FP16, kind="ExternalInput").ap()
    # host pre-tiles V-augmented to [BPC, 128, NKT, DA] so the DMA is contiguous
    va_d = nc.dram_tensor("va", [BPC, 128, NKT, DA], FP16, kind="ExternalInput").ap()
    # device writes [BPC, NH, 128, 8*D] contiguously; host re-tiles to [B, S, D]
    out_d = nc.dram_tensor("out", [BPC, NH, 128, 8 * D], FP16, kind="ExternalOutput").ap()

    with tile.TileContext(nc) as tc:
        with (
            tc.tile_pool(name="io", bufs=1) as io,
            tc.tile_pool(name="const", bufs=1) as const,
            tc.tile_pool(name="pt", bufs=8) as ptp,
            tc.tile_pool(name="csb", bufs=2) as csbp,
            tc.tile_pool(name="outsb", bufs=2) as outp,
            tc.tile_pool(name="small", bufs=2) as small,
            tc.tile_pool(name="dvet", bufs=2) as dvet,
            tc.tile_pool(name="scps", bufs=3, space="PSUM") as scps,
            tc.tile_pool(name="cxps", bufs=1, space="PSUM") as cxps,
        ):

            def dve_exp(sc, pt):
                """pt = exp(sc/8) on the Vector engine via fp16 2^y bits.

                First op reads PSUM fp32 (1x) and frees sc; the rest are
                fp16 SBUF: tensor_scalar at 4x, tensor_tensor at 2x.
                """
                y = dvet.tile([128, HQ], FP16, name="y16")
                nc.vector.tensor_scalar_mul(y, sc, LOG2E_8)
                t = dvet.tile([128, HQ], FP16, name="t16")
                nc.vector.tensor_scalar_add(t, y, MAGIC)
                n = dvet.tile([128, HQ], FP16, name="n16")
                nc.vector.tensor_scalar_sub(n, t, MAGIC)
                s16 = dvet.tile([128, HQ], I16, name="s16")
                nc.vector.tensor_scalar(
                    s16, t, 1024.0, SBIAS,
                    op0=mybir.AluOpType.mult, op1=mybir.AluOpType.add,
                )
                f = dvet.tile([128, HQ], FP16, name="f16")
                nc.vector.tensor_tensor(f, y, n, op=mybir.AluOpType.subtract)
                a = dvet.tile([128, HQ], FP16, name="a16")
                nc.vector.tensor_scalar(
                    a, f, C2, C1,
                    op0=mybir.AluOpType.mult, op1=mybir.AluOpType.add,
                )
                nc.vector.tensor_tensor(a, a, f, op=mybir.AluOpType.mult)
                nc.vector.tensor_scalar_add(a, a, C0)
                nc.vector.tensor_tensor(
                    pt, a, s16.bitcast(FP16), op=mybir.AluOpType.mult
                )

            def drain(cx, b, h):
                # split into small closures; one is emitted per k-step so
                # the work interleaves with the next half's steps
                state = {}

                def start():
                    csb = csbp.tile([DA, HQ], FP16, name="csb")
                    nc.vector.tensor_copy(csb, cx)
                    state["csb"] = csb
                    state["out_sb"] = outp.tile([128, 8 * D], FP16, name="out_sb")
                    state["ct"] = csbp.tile([128, 8 * DA], FP16, name="ctT")
                    state["r8"] = small.tile([128, 8], F32, name="r8")

                def chunk(c):
                    def emit():
                        # xbar transpose [DA, 128] chunk -> [128, DA]
                        nc.sync.dma_start_transpose(
                            state["ct"][:, c * DA : (c + 1) * DA],
                            state["csb"][:, c * 128 : (c + 1) * 128],
                        )

                    return emit

                def norm():
                    ct, r8, out_sb = state["ct"], state["r8"], state["out_sb"]
                    # denominator sits at col D of each DA-wide chunk
                    nc.vector.reciprocal(r8, ct[:, D :: DA][:, 0:8])
                    for c in range(8):
                        nc.vector.tensor_scalar_mul(
                            out_sb[:, c * D : (c + 1) * D],
                            ct[:, c * DA : c * DA + D],
                            r8[:, c : c + 1],
                        )

                def store():
                    nc.gpsimd.dma_start(out=out_d[b, h], in_=state["out_sb"])

                return [start] + [chunk(c) for c in range(8)] + [norm, store]

            def body():
                pending = []  # deferred drain closures, one popped per step

                # prefetch all four batches up-front: qt/kt on the SP HWDGE
                # queue (needed first), va on the idle Pool SWDGE queue
                qts, kts, vas = [], [], []
                for b in range(BPC):
                    qt_sb = io.tile([D, S], FP16, name=f"qt{b}")
                    kt_sb = io.tile([D, S], FP16, name=f"kt{b}")
                    va_sb = io.tile([128, NKT, DA], FP16, name=f"va{b}")
                    nc.sync.dma_start(out=kt_sb, in_=kt_d[b])
                    nc.sync.dma_start(out=qt_sb, in_=qt_d[b])
                    nc.gpsimd.dma_start(out=va_sb, in_=va_d[b])
                    qts.append(qt_sb)
                    kts.append(kt_sb)
                    vas.append(va_sb)

                av_due = []  # (due_gstep, k, pt, cx, va_sb, emitted)
                gstep = [0]

                def flush_av(final=False):
                    rest = []
                    due_now = []
                    for item in av_due:
                        if final or item[0] <= gstep[0]:
                            due_now.append(item)
                        else:
                            rest.append(item)
                    av_due[:] = rest
                    for _, k, pt, cx, va_sb, emitted, dr in due_now:
                        # matmul out must stay within one PSUM bank:
                        # emit per 512-col half
                        is_start = emitted[0] == 0
                        is_stop = emitted[0] == NKT - 1
                        emitted[0] += 1
                        for j in range(2):
                            nc.tensor.matmul(
                                cx[:, j * 512 : (j + 1) * 512],
                                lhsT=va_sb[:, k, :],
                                rhs=pt[:, j * 512 : (j + 1) * 512],
                                start=is_start,
                                stop=is_stop,
                                skip_group_check=True,
                            )
                        if emitted[0] == NKT:
                            # all AV writers of this cx are now registered;
                            # only now is the drain's cx->SBUF copy safe to emit
                            pending.extend(dr())

                for b in range(BPC):
                    qt_sb, kt_sb, va_sb = qts[b], kts[b], vas[b]
                    for h in range(NH):
                        cx = cxps.tile([DA, HQ], F32)
                        q0 = h * HQ
                        emitted = [0]
                        for k in range(NKT):
                            sc = scps.tile([128, HQ], F32)
                            for j in range(2):
                                nc.tensor.matmul(
                                    sc[:, j * 512 : (j + 1) * 512],
                                    lhsT=kt_sb[:, k * 128 : (k + 1) * 128],
                                    rhs=qt_sb[:, q0 + j * 512 : q0 + (j + 1) * 512],
                                    start=True,
                                    stop=True,
                                )
                            flush_av()
                            pt = ptp.tile([128, HQ], FP16)
                            dks = dve_steps(b, h)
                            delay = DVE_AV_DELAY if k in dks else 1
                            if k in dks:
                                dve_exp(sc, pt)
                            else:
                                nc.scalar.activation(
                                    out=pt,
                                    in_=sc,
                                    func=mybir.ActivationFunctionType.Exp,
                                    scale=0.125,
                                )
                            av_due.append(
                                (gstep[0] + delay, k, pt, cx, va_sb, emitted,
                                 (lambda cx=cx, b=b, h=h: drain(cx, b, h)))
                            )
                            gstep[0] += 1
                            if pending:
                                pending.pop(0)()
                flush_av(final=True)
                for p in pending:
                    p()

            if reps == 1:
                body()
            else:
                with tc.For_i(
                    0,
                    reps,
                    1,
                    hint_engines=(
                        mybir.EngineType.PE,
                        mybir.EngineType.Activation,
                        mybir.EngineType.DVE,
                        mybir.EngineType.SP,
                    ),
                ):
                    body()

    nc.compile()
    _cache[reps] = nc
    return nc


def _prep_core_inputs(query, key, value, core):
    sl = slice(core * BPC, (core + 1) * BPC)
    qT = np.ascontiguousarray(query[sl].transpose(0, 2, 1)).astype(np.float16)
    kT = np.ascontiguousarray(key[sl].transpose(0, 2, 1)).astype(np.float16)
    v16 = value[sl].astype(np.float16)
    ones = np.ones((BPC, S, DA - D), dtype=np.float16)
    va = np.concatenate([v16, ones], axis=2)
    # [BPC, S, DA] -> [BPC, 128, NKT, DA]: row s = n*128 + p lives at [p, n]
    va_t = np.ascontiguousarray(va.reshape(BPC, NKT, 128, DA).transpose(0, 2, 1, 3))
    return {
        "qt": qT,
        "kt": kT,
        "va": va_t,
    }


def run(query, key, value, trace=False):
    nc = _build()
    query = np.asarray(query, dtype=np.float32)
    key = np.asarray(key, dtype=np.float32)
    value = np.asarray(value, dtype=np.float32)
    in_maps = [_prep_core_inputs(query, key, value, c) for c in range(NCORES)]
    res = run_bass_kernel_spmd(nc, in_maps, core_ids=list(range(NCORES)))
    outs = []
    for c in range(NCORES):
        o = np.asarray(res.results[c]["out"])  # [BPC, NH, 128, 8*D]
        o = o.reshape(BPC, NH, 128, 8, D).transpose(0, 1, 3, 2, 4).reshape(BPC, S, D)
        outs.append(o)
    return np.concatenate(outs, axis=0).astype(np.float32), res


def kernel(query, key, value):
    out, _ = run(query, key, value)
    return out

